# revision 1
# baseline (speedup 1.0000x reference)
"""GQA attention (B=2, L=2048, HID=2048, 32 Q heads / 8 KV heads) on 8 TRN2 cores.

Sharding: data-parallel on batch (2) x tensor-parallel on heads (4).
Core c: batch b = c//4, TP rank r = c%4 owns q heads {8r..8r+7} (whole GQA
groups: kv heads 2r, 2r+1). bf16 TensorEngine compute, fp32 PSUM, fp32
softmax statistics. Per-core pipeline:
  1. KV proj (streamed kvT pieces, padded k-tiles skipped entirely):
     kT [128, Lk] and per-tile v [128, 130] (+ones cols for the softmax
     denominator ride-along).
  2. Q proj from SBUF-resident qT: QT[pr] = [128, L] bf16, head-pair rows.
  3. Attention per (j, pr): software-pipelined QK -> exp -> (band mul) -> PV
     with the QK of step i+1 emitted before PV of step i so the PE never
     stalls behind the scalar-engine exp. Diagonal band tiles are
     column-trimmed (leading fully-masked q columns skipped in exp/PV/QK).
  4. Output projection in ReduceScatter form: each core computes partial
     outT[2048 f, 512 q] from its OWN heads only (no gather wait), bias
     folded into the PSUM unload via a per-core zero-padded bias input,
     then a bf16 ReduceScatter(add) per q-chunk produces the final
     [512 f, 512 q] slab owned by this core.
Host assembles [2, 2048, 2048] f32 from per-core [4][512, 512] bf16 slabs.

Mask handling is input-driven: blocks are classified all-masked (skipped),
all-zero (no mask op), or band (exp(mask) multiplied into exp(scores));
band tiles additionally get a leading-dead-column trim q0.
"""

import numpy as np
import ml_dtypes
import concourse.bass as bass
import concourse.mybir as mybir
import concourse.tile as tile
from concourse import bacc
from concourse.bass_utils import run_bass_kernel_spmd

F32 = mybir.dt.float32
BF16 = mybir.dt.bfloat16
AF = mybir.ActivationFunctionType
NPBF16 = ml_dtypes.bfloat16

B, L, HID = 2, 2048, 2048
NH, D, NKV = 32, 64, 8
SCALE = 0.125
N_CORES = 8
TPR = 4          # TP ranks per batch group
NPAIR = 4        # head pairs per core
LQC = 512        # q chunk (PSUM-bank sized)
NJ = L // LQC    # 4
KT = 128         # k-position tile
NI = L // KT     # 16
NKC = HID // 128  # 16 contraction chunks
NEG_THRESH = -1.0e8

_graph_cache = {}
last_results = None  # BassKernelResults of the most recent run (for test harness)
DEBUG = False  # adds debug dram outputs (dbg.py)


def _classify_blocks(eff_masks):
    """eff_masks: list of B arrays [L, L] (q, k). Returns (live, band_list,
    trim) where live[j] lists live k-tiles for q-chunk j, band_list orders
    blocks needing explicit mask values, and trim[(j, i)] is the count of
    leading q-columns of the transposed block that are fully masked."""
    live = {}
    band_list = []
    trim = {}
    for j in range(NJ):
        lv = []
        for i in range(NI):
            subs = [m[j * LQC:(j + 1) * LQC, i * KT:(i + 1) * KT] for m in eff_masks]
            if all((s <= NEG_THRESH).all() for s in subs):
                continue  # fully masked in every batch
            lv.append(i)
            # leading q-columns (rows of the [LQC, KT] block) dead in all b
            dead_q = np.logical_and.reduce(
                [(s <= NEG_THRESH).all(axis=1) for s in subs])
            q0 = 0
            while q0 < LQC and dead_q[q0]:
                q0 += 1
            q0 &= ~127  # keep alignment coarse; only full-128 steps trimmed
            trim[(j, i)] = q0
            if not all((s == 0.0).all() for s in subs):
                band_list.append((j, i))
        live[j] = lv
    return live, band_list, trim


def _build_graph(live_key, band_key, trim_key, live_k):
    key = (live_key, band_key, trim_key, tuple(live_k), DEBUG)
    if key in _graph_cache:
        return _graph_cache[key]

    live = {j: list(lv) for j, lv in live_key}
    band_list = list(band_key)
    trim = dict(trim_key)
    band_idx = {ji: n for n, ji in enumerate(band_list)}
    nb = max(1, len(band_list))
    nt = len(live_k)              # live k tiles
    Lk = nt * KT
    pos_of = {i: t for t, i in enumerate(live_k)}
    NSK = (Lk + 511) // 512       # kT column sub-blocks

    nc = bacc.Bacc("TRN2", target_bir_lowering=False, debug=False,
                   num_devices=N_CORES)

    # host-prelayouted inputs: [128, ...] sbuf-shaped flat rows
    qT = nc.dram_tensor("qT", [128, NKC * L], BF16, kind="ExternalInput")
    kvT = nc.dram_tensor("kvT", [128, NKC * Lk], BF16, kind="ExternalInput")
    wq = nc.dram_tensor("wq", [128, NKC * 512], BF16, kind="ExternalInput")
    # wkv: chunk k -> cols [256k:256k+128] = wk, [256k+128:256k+256] = wv
    wkv = nc.dram_tensor("wkv", [128, NKC * 256], BF16, kind="ExternalInput")
    wo = nc.dram_tensor("wo", [128, NPAIR * HID], BF16, kind="ExternalInput")
    # consts_bf: cols 0:128 ones; row 0 cols 128:640 = bv4
    consts_bf = nc.dram_tensor("consts_bf", [128, 640], BF16,
                               kind="ExternalInput")
    # consts_f32: cols 0:4 bq, col 4 bk, cols 5:21 bo_pad
    consts_f32 = nc.dram_tensor("consts_f32", [128, 21], F32,
                                kind="ExternalInput")
    band = nc.dram_tensor("band", [128, nb * LQC], BF16, kind="ExternalInput")

    rs_warm_in = nc.dram_tensor("rs_warm_in", [128, 16], BF16)
    rs_warm_out = nc.dram_tensor("rs_warm_out", [32, 16], BF16)
    # last chunk is q-halved (two smaller ReduceScatters to shrink the tail)
    rs_in = [nc.dram_tensor(f"rs_in{j}", [HID, LQC], BF16)
             for j in range(NJ - 1)]
    rs_out = [nc.dram_tensor(f"rs_out{j}", [512, LQC], BF16)
              for j in range(NJ - 1)]
    rs_in3 = [nc.dram_tensor(f"rs_in3_{h}", [HID, LQC // 2], BF16)
              for h in range(2)]
    rs_out3 = [nc.dram_tensor(f"rs_out3_{h}", [512, LQC // 2], BF16)
               for h in range(2)]
    out_ext = [nc.dram_tensor(f"out{j}", [512, LQC], BF16,
                              kind="ExternalOutput") for j in range(NJ)]
    groups = [[0, 1, 2, 3], [4, 5, 6, 7]]
    dbg = {}
    if DEBUG:
        dbg["kT"] = nc.dram_tensor("dbg_kT", [128, Lk], BF16,
                                   kind="ExternalOutput")
        dbg["v0"] = nc.dram_tensor("dbg_v0", [128, 130], BF16,
                                   kind="ExternalOutput")
        dbg["QT0"] = nc.dram_tensor("dbg_QT0", [128, L], BF16,
                                    kind="ExternalOutput")
        dbg["ua"] = nc.dram_tensor("dbg_ua", [65, 512], F32,
                                   kind="ExternalOutput")
        dbg["ub"] = nc.dram_tensor("dbg_ub", [65, 512], F32,
                                   kind="ExternalOutput")
        dbg["rba"] = nc.dram_tensor("dbg_rba", [64, 512], F32,
                                    kind="ExternalOutput")
        dbg["at0"] = nc.dram_tensor("dbg_at0", [128, 512], BF16,
                                    kind="ExternalOutput")
        dbg["rsin0"] = nc.dram_tensor("dbg_rsin0", [HID, LQC], BF16,
                                      kind="ExternalOutput")

    with tile.TileContext(nc) as tc:
        with tc.tile_pool(name="persist", bufs=1) as persist:
            # DMA plan (issue cost ~2.5us/DMA per queue -> few, big, ordered):
            #  sync:   consts_bf, kv piece0, consts_f32, kv piece2, qtA
            #  scalar: wkv, kv piece1, kv piece3, wq, qtB, wo, band
            #  gpsimd: collective warmup; later atb shifts + RS triggers
            # -- allocate all persistent tiles up front
            # NOTE tile deps are whole-tile: anything streamed in pieces that
            # must be consumed incrementally gets its own tile per piece.
            cbf_sb = persist.tile([128, 640], BF16, tag="cbf")
            wkv_sb = persist.tile([128, NKC * 256], BF16, tag="wkv")
            cf_sb = persist.tile([128, 21], F32, tag="cf")
            wq_sb = persist.tile([128, NKC * 512], BF16, tag="wq")
            wo_sb = persist.tile([128, NPAIR * HID], BF16, tag="wo")
            band_sb = persist.tile([128, nb * LQC], BF16, tag="band")
            kT_sb = persist.tile([128, Lk], BF16, tag="kT")
            v_sb = [persist.tile([128, 130], BF16, tag=f"v{t}", name=f"v{t}")
                    for t in range(nt)]
            QT_sb = [[persist.tile([128, 1024], BF16, tag=f"qt{m}{jp}",
                                   name=f"qt{m}{jp}") for jp in range(2)]
                     for m in range(NPAIR)]
            ones_sb = cbf_sb[:, 0:128]
            bv4_sb = cbf_sb[0:1, 128:640]
            bq_sb = cf_sb[:, 0:4]
            bk_sb = cf_sb[:, 4:5]
            bo_sb = cf_sb[:, 5:21]

            nc.sync.dma_start(cbf_sb[:], consts_bf[:])
            nc.scalar.dma_start(wkv_sb[:], wkv[:])
            for t in range(nt):
                nc.vector.tensor_copy(v_sb[t][:, 64:65], ones_sb[:, 0:1])
                nc.vector.tensor_copy(v_sb[t][:, 129:130], ones_sb[:, 0:1])

            # qt staging is read by the Q jp1 sub-eras inside attention
            # scope 1, so it lives at persist level.
            qtp = [[persist.tile([128, 8 * 1024], BF16, tag=f"qtp{jp}{h}",
                                 name=f"qtp{jp}{h}") for h in range(2)]
                   for jp in range(2)]

            # ---- KV projection (kvch era-scoped; queue-ordered DMAs)
            with (
                tc.tile_pool(name="kv_stream", bufs=1) as kvs,
                tc.tile_pool(name="kv_psum", bufs=1, space="PSUM") as kvp,
            ):
                # ALL critical input transfers go on the sync ring in exact
                # consumption order — SDMA round-robins *rings* at packet
                # granularity, so priority only exists within one ring.
                kvpc = [kvs.tile([128, 4 * Lk], BF16, tag=f"kvpc{p}",
                                 name=f"kvpc{p}") for p in range(4)]
                for p in range(4):
                    nc.sync.dma_start(kvpc[p][:],
                                      kvT[:, 4 * p * Lk:4 * (p + 1) * Lk])
                nc.sync.dma_start(cf_sb[:], consts_f32[:])
                nc.sync.dma_start(wq_sb[:], wq[:])
                for jp in range(2):
                    for h in range(2):
                        nc.sync.dma_start(
                            qtp[jp][h][:],
                            qT[:, (2 * jp + h) * 8192:(2 * jp + h + 1) * 8192])
                # band/wo after the hot inputs on the same ring (needed later)
                nc.sync.dma_start(band_sb[:], band[:])
                nc.sync.dma_start(wo_sb[:], wo[:])
                nc.gpsimd.dma_start(rs_warm_in[:], consts_bf[:, 0:16])
                nc.gpsimd.collective_compute(
                    "ReduceScatter", mybir.AluOpType.add,
                    replica_groups=groups,
                    ins=[rs_warm_in[:]], outs=[rs_warm_out[:]])
                wids = [min(512, Lk - 512 * s) for s in range(NSK)]
                psk = [kvp.tile([128, wids[s]], F32, tag=f"psk{s}",
                                name=f"psk{s}") for s in range(NSK)]
                psv = [kvp.tile([128, wids[s]], F32, tag=f"psv{s}",
                                name=f"psv{s}") for s in range(NSK)]
                for s in range(NSK):
                    nc.tensor.matmul(psv[s][:], ones_sb[0:1, :],
                                     bv4_sb[:, 0:wids[s]], start=True,
                                     stop=False, skip_group_check=True)
                for k in range(NKC):
                    kv_ch = kvpc[k // 4]
                    off = (k % 4) * Lk
                    for s in range(NSK):
                        nc.tensor.matmul(
                            psk[s][:], wkv_sb[:, 256 * k:256 * k + 128],
                            kv_ch[:, off + 512 * s:off + 512 * s + wids[s]],
                            start=(k == 0), stop=(k == NKC - 1))
                    for t in range(nt):
                        s, col = t // 4, t % 4
                        nc.tensor.matmul(
                            psv[s][:, 128 * col:128 * (col + 1)],
                            kv_ch[:, off + 128 * t:off + 128 * (t + 1)],
                            wkv_sb[:, 256 * k + 128:256 * (k + 1)],
                            start=False, stop=(k == NKC - 1),
                            skip_group_check=True)
                for s in range(NSK):
                    nc.scalar.activation(kT_sb[:, 512 * s:512 * s + wids[s]],
                                         psk[s][:], AF.Identity, bias=bk_sb[:])
                for t in range(nt):
                    s, col = t // 4, t % 4
                    nc.scalar.copy(v_sb[t][:, 0:64],
                                   psv[s][:, 128 * col:128 * col + 64])
                    nc.vector.tensor_copy(
                        v_sb[t][:, 65:129],
                        psv[s][:, 128 * col + 64:128 * (col + 1)])
                if DEBUG:
                    nc.sync.dma_start(dbg["kT"][:], kT_sb[:])
                    nc.sync.dma_start(dbg["v0"][:], v_sb[0][:])

            # ---- Q projection from resident qT (jp-major layout).
            # jp=1 is interleaved into attention chunk 0 (scope 1 below).
            with tc.tile_pool(name="q_psum", bufs=1, space="PSUM") as qp:
                for jp in range(1):
                    psq = [qp.tile([128, 512], F32, tag=f"psq{n}",
                                   name=f"psq{n}") for n in range(8)]
                    for k in range(NKC):
                        qch = qtp[jp][k // 8]
                        off = (k % 8) * 1024
                        for m in range(NPAIR):
                            for jj in range(2):
                                nc.tensor.matmul(
                                    psq[4 * jj + m][:],
                                    wq_sb[:, 512 * k + 128 * m:
                                             512 * k + 128 * (m + 1)],
                                    qch[:, off + 512 * jj:off + 512 * (jj + 1)],
                                    start=(k == 0), stop=(k == NKC - 1))
                    for jj in range(2):
                        for m in range(NPAIR):
                            nc.scalar.activation(
                                QT_sb[m][jp][:, 512 * jj:512 * (jj + 1)],
                                psq[4 * jj + m][:], AF.Identity,
                                bias=bq_sb[:, m:m + 1])
                if DEBUG:
                    nc.sync.dma_start(dbg["QT0"][:, 0:1024], QT_sb[0][0][:])
                    nc.sync.dma_start(dbg["QT0"][:, 1024:2048], QT_sb[0][1][:])

            # ---- Attention + ReduceScatter output projection.
            # Two pool scopes: scope 1 runs chunk 0 with the Q jp=1
            # sub-eras interleaved (PSUM: qk 4 + pv 2 + qsub 2); scope 2
            # runs chunks 1..3 with oproj (PSUM: qk 4 + pv 2 + pso 2).
            with tc.tile_pool(name="at_pool", bufs=2) as at_pool:
                P = {}
                pending = []  # (jc, f, qh, at_tiles) oproj work items

                ob_cur = [None]

                def oproj_tile(jc, f, qh, ats):
                    # qh None: full 512 q (chunks 0..2); else 256-q half of j=3
                    w = 512 if qh is None else 256
                    q0, dst = (0, rs_in[jc]) if qh is None else \
                        (256 * qh, rs_in3[qh])
                    pso = P["o_psum"].tile([128, 512], F32, tag="pso")
                    for pr in range(NPAIR):
                        nc.tensor.matmul(
                            pso[:, 0:w],
                            wo_sb[:, HID * pr + 128 * f:HID * pr + 128 * (f + 1)],
                            ats[pr][:, q0:q0 + w], start=(pr == 0),
                            stop=(pr == 3), skip_group_check=True)
                    # pack 4 f-tiles into one ob tile -> one staging DMA
                    if f % 4 == 0:
                        obt = P["ob_pool"].tile([128, 4 * w], BF16, tag=f"ob{w}",
                                           name=f"ob{w}")
                        ob_cur[0] = obt
                    ob = ob_cur[0]
                    sl = ob[:, (f % 4) * w:(f % 4 + 1) * w]
                    if f % 2 == 0:
                        nc.scalar.activation(sl, pso[:, 0:w], AF.Identity,
                                             bias=bo_sb[:, f:f + 1])
                    else:
                        nc.vector.tensor_scalar_add(sl, pso[:, 0:w],
                                                    bo_sb[:, f:f + 1])
                    if f % 4 == 3:
                        g = f // 4
                        dst_ap = dst[512 * g:512 * (g + 1), :].rearrange(
                            "(a p) c -> p a c", p=128)
                        src_ap = ob[:].rearrange("p (a c) -> p a c", a=4)
                        nc.sync.dma_start(dst_ap, src_ap)

                def pop_pending():
                    if pending:
                        jc, f, qh, ats = pending.pop(0)
                        oproj_tile(jc, f, qh, ats)
                        if f == 15:
                            if DEBUG and jc == 0:
                                nc.sync.dma_start(dbg["rsin0"][:], rs_in[0][:])
                            cin = rs_in[jc] if qh is None else rs_in3[qh]
                            cout = rs_out[jc] if qh is None else rs_out3[qh]
                            nc.gpsimd.collective_compute(
                                "ReduceScatter", mybir.AluOpType.add,
                                replica_groups=groups,
                                ins=[cin[:]], outs=[cout[:]])

                def attn_block(j, pr, ats):
                    lv = live[j]
                    nlast = len(lv) - 1
                    pva = P["pv_psum"].tile([65, 512], F32, tag="pva")
                    pvb = P["pv_psum"].tile([65, 512], F32, tag="pvb")
                    npop = 4
                    pts = []
                    for n, i in enumerate(lv):
                        t = pos_of[i]
                        q0 = trim.get((j, i), 0)
                        qt_t = QT_sb[pr][j // 2]
                        qoff = 512 * (j % 2)
                        ps = P["qk_psum"].tile([128, 1024], F32, tag="qk")
                        nc.tensor.matmul(
                            ps[:, q0:512],
                            kT_sb[0:64, 128 * t:128 * (t + 1)],
                            qt_t[0:64, qoff + q0:qoff + 512],
                            start=True, stop=True, skip_group_check=True)
                        nc.tensor.matmul(
                            ps[:, 512 + q0:1024],
                            kT_sb[64:128, 128 * t:128 * (t + 1)],
                            qt_t[64:128, qoff + q0:qoff + 512],
                            start=True, stop=True, skip_group_check=True)
                        pt = P["pt_pool"].tile([128, 1024], BF16, tag="pt")
                        if q0 == 0:
                            nc.scalar.activation(pt[:], ps[:], AF.Exp)
                        else:
                            nc.scalar.activation(pt[:, q0:512],
                                                 ps[:, q0:512], AF.Exp)
                            nc.scalar.activation(pt[:, 512 + q0:1024],
                                                 ps[:, 512 + q0:1024], AF.Exp)
                        if (j, i) in band_idx:
                            bcol = band_idx[(j, i)] * LQC
                            nc.vector.tensor_mul(
                                pt[:, q0:512], pt[:, q0:512],
                                band_sb[:, bcol + q0:bcol + 512])
                            nc.vector.tensor_mul(
                                pt[:, 512 + q0:1024], pt[:, 512 + q0:1024],
                                band_sb[:, bcol + q0:bcol + 512])
                        pts.append((pt, q0))
                        # software pipeline: PV lags QK/exp by one step
                        if n > 0:
                            ptp, q0p = pts[n - 1]
                            tp = pos_of[lv[n - 1]]
                            nc.tensor.matmul(
                                pva[:, q0p:512], v_sb[tp][:, 0:65],
                                ptp[:, q0p:512], start=(n - 1 == 0),
                                stop=False, skip_group_check=True)
                            nc.tensor.matmul(
                                pvb[:, q0p:512], v_sb[tp][:, 65:130],
                                ptp[:, 512 + q0p:1024], start=(n - 1 == 0),
                                stop=False, skip_group_check=True)
                        if npop and n in (2, 4, 6, 7):
                            pop_pending()
                            npop -= 1
                    ptp, q0p = pts[nlast]
                    tp = pos_of[lv[nlast]]
                    nc.tensor.matmul(
                        pva[:, q0p:512], v_sb[tp][:, 0:65], ptp[:, q0p:512],
                        start=(nlast == 0), stop=True, skip_group_check=True)
                    nc.tensor.matmul(
                        pvb[:, q0p:512], v_sb[tp][:, 65:130],
                        ptp[:, 512 + q0p:1024], start=(nlast == 0), stop=True,
                        skip_group_check=True)
                    while npop:
                        pop_pending()
                        npop -= 1
                    # unload + normalize
                    ua = P["ua_pool"].tile([65, 512], F32, tag="ua")
                    ub = P["ua_pool"].tile([65, 512], F32, tag="ub")
                    nc.scalar.copy(ua[:], pva[:])
                    nc.vector.tensor_copy(ub[:], pvb[:])
                    # den rows to partition 0 (DVE cannot partition-shift
                    # SBUF->SBUF; PSUM row reads to p0 are fine per baseline)
                    rsa = P["rc_pool"].tile([1, 512], F32, tag="rsa")
                    rsb = P["rc_pool"].tile([1, 512], F32, tag="rsb")
                    nc.vector.tensor_copy(rsa[:], pva[64:65, :])
                    nc.vector.tensor_copy(rsb[:], pvb[64:65, :])
                    ra = P["rc_pool"].tile([1, 512], F32, tag="ra")
                    rb = P["rc_pool"].tile([1, 512], F32, tag="rb")
                    nc.vector.reciprocal_approx_fast(out=ra[:], in_=rsa[:])
                    nc.vector.reciprocal_approx_fast(out=rb[:], in_=rsb[:])
                    rba = P["rc_pool"].tile([64, 512], F32, tag="rba")
                    rbb = P["rc_pool"].tile([64, 512], F32, tag="rbb")
                    nc.gpsimd.partition_broadcast(rba[:], ra[:])
                    nc.gpsimd.partition_broadcast(rbb[:], rb[:])
                    nc.vector.tensor_mul(ats[pr][0:64, :], ua[0:64, :], rba[:])
                    atb = P["rc_pool"].tile([64, 512], BF16, tag="atb")
                    nc.vector.tensor_mul(atb[:], ub[0:64, :], rbb[:])
                    # NOT on gpsimd: a queued collective blocks that queue
                    nc.scalar.dma_start(ats[pr][64:128, :], atb[:])
                    if DEBUG and j == 0 and pr == 0:
                        nc.sync.dma_start(dbg["ua"][:], ua[:])
                        nc.sync.dma_start(dbg["ub"][:], ub[:])
                        nc.sync.dma_start(dbg["rba"][:], rba[:])
                        nc.sync.dma_start(dbg["at0"][:], ats[0][:])

                def q_sub_era(s, qsub):
                    jj, mp = s // 2, s % 2
                    psq = qsub.tile([128, 1024], F32, tag="qsub")
                    for k in range(NKC):
                        qch = qtp[1][k // 8]
                        off = (k % 8) * 1024 + 512 * jj
                        for mi in range(2):
                            m = 2 * mp + mi
                            nc.tensor.matmul(
                                psq[:, 512 * mi:512 * (mi + 1)],
                                wq_sb[:, 512 * k + 128 * m:
                                         512 * k + 128 * (m + 1)],
                                qch[:, off:off + 512],
                                start=(k == 0), stop=(k == NKC - 1))
                    for mi in range(2):
                        m = 2 * mp + mi
                        nc.scalar.activation(
                            QT_sb[m][1][:, 512 * jj:512 * (jj + 1)],
                            psq[:, 512 * mi:512 * (mi + 1)], AF.Identity,
                            bias=bq_sb[:, m:m + 1])

                ats0 = [at_pool.tile([128, 512], BF16, tag=f"at{pr}",
                                     name=f"at{pr}_0")
                        for pr in range(NPAIR)]
                with (
                    tc.tile_pool(name="pt1", bufs=3) as _pt1,
                    tc.tile_pool(name="ua1", bufs=2) as _ua1,
                    tc.tile_pool(name="rc1", bufs=2) as _rc1,
                    tc.tile_pool(name="qk1", bufs=2, space="PSUM") as _qk1,
                    tc.tile_pool(name="pv1", bufs=1, space="PSUM") as _pv1,
                    tc.tile_pool(name="qsub", bufs=1, space="PSUM") as _qs,
                ):
                    P.update(pt_pool=_pt1, ua_pool=_ua1, rc_pool=_rc1,
                             qk_psum=_qk1, pv_psum=_pv1)
                    for pr in range(NPAIR):
                        attn_block(0, pr, ats0)
                        q_sub_era(pr, _qs)
                for f in range(16):
                    pending.append((0, f, None, ats0))

                with (
                    tc.tile_pool(name="pt2", bufs=3) as _pt2,
                    tc.tile_pool(name="ua2", bufs=2) as _ua2,
                    tc.tile_pool(name="rc2", bufs=2) as _rc2,
                    tc.tile_pool(name="ob2", bufs=2) as _ob2,
                    tc.tile_pool(name="qk2", bufs=2, space="PSUM") as _qk2,
                    tc.tile_pool(name="pv2", bufs=1, space="PSUM") as _pv2,
                    tc.tile_pool(name="o_ps", bufs=2, space="PSUM") as _ops,
                ):
                    P.update(pt_pool=_pt2, ua_pool=_ua2, rc_pool=_rc2,
                             ob_pool=_ob2, qk_psum=_qk2, pv_psum=_pv2,
                             o_psum=_ops)
                    for j in range(1, NJ):
                        ats = [at_pool.tile([128, 512], BF16, tag=f"at{pr}",
                                            name=f"at{pr}_{j}")
                               for pr in range(NPAIR)]
                        for pr in range(NPAIR):
                            attn_block(j, pr, ats)
                        if j < NJ - 1:
                            for f in range(16):
                                pending.append((j, f, None, ats))
                        else:
                            for qh in range(2):
                                for f in range(16):
                                    pending.append((j, f, qh, ats))
                    while pending:
                        pop_pending()
                    # final output copies (off the hot critical path)
                    for jc in range(NJ - 1):
                        nc.sync.dma_start(out_ext[jc][:], rs_out[jc][:])
                    for qh in range(2):
                        nc.sync.dma_start(
                            out_ext[3][:, 256 * qh:256 * (qh + 1)],
                            rs_out3[qh][:])

    nc.compile()
    _graph_cache[key] = nc
    return nc


def _prelayout(a, width):
    """[NKC*128, width] row-major -> [128, NKC*width] sbuf layout."""
    return np.ascontiguousarray(
        a.reshape(NKC, 128, width).transpose(1, 0, 2).reshape(128, NKC * width))


def kernel(query, kv, Wq, bq, Wkv, bkv, Wo, bo, attn_mask, key_padding_mask):
    global last_results
    query = np.asarray(query, np.float32)
    kv = np.asarray(kv, np.float32)
    Wq = np.asarray(Wq, np.float32)
    bq = np.asarray(bq, np.float32)
    Wkv = np.asarray(Wkv, np.float32)
    bkv = np.asarray(bkv, np.float32)
    Wo = np.asarray(Wo, np.float32)
    bo = np.asarray(bo, np.float32)
    attn_mask = np.asarray(attn_mask, np.float32)
    kpm = np.asarray(key_padding_mask)

    eff = [attn_mask + np.where(kpm[b], np.float32(-1e9), np.float32(0.0))[None, :]
           for b in range(B)]
    live, band_list, trim = _classify_blocks(eff)
    live_k = sorted({i for lv in live.values() for i in lv})
    live_key = tuple((j, tuple(lv)) for j, lv in sorted(live.items()))
    band_key = tuple(band_list)
    trim_key = tuple(sorted(trim.items()))

    nc = _build_graph(live_key, band_key, trim_key, live_k)

    nt = len(live_k)
    Lk = nt * KT

    # Host-side shard prep. qT jp-major: col = jp*16*1024 + k*1024 + c
    qTh = [np.ascontiguousarray(
        query[b].T.astype(NPBF16).reshape(NKC, 128, 2, 1024)
        .transpose(2, 1, 0, 3).reshape(2, 128, NKC * 1024)
        .transpose(1, 0, 2).reshape(128, NKC * L)) for b in range(B)]
    kvTsel = [np.ascontiguousarray(
        kv[b].T.astype(NPBF16)
        .reshape(HID, NI, KT)[:, live_k, :].reshape(HID, Lk)) for b in range(B)]
    kvTh = [_prelayout(k_, Lk) for k_ in kvTsel]
    nb = max(1, len(band_list))
    bandh = []
    with np.errstate(over="ignore", under="ignore"):
        for b in range(B):
            if band_list:
                bandh.append(np.ascontiguousarray(np.concatenate(
                    [np.exp(eff[b][j * LQC:(j + 1) * LQC,
                                   i * KT:(i + 1) * KT].T)
                     for (j, i) in band_list], axis=1).astype(NPBF16)))
            else:
                bandh.append(np.zeros((KT, nb * LQC), NPBF16))
    ones_h = np.ones((128, 128), NPBF16)

    Wq_h = Wq.reshape(HID, NH, D)
    bq_h = bq.reshape(NH, D)
    Wo_h = Wo.reshape(NH, D, HID)

    in_maps = []
    for c in range(N_CORES):
        b, r = c // TPR, c % TPR
        heads_q = [8 * r + pr + 4 * e for pr in range(NPAIR) for e in range(2)]
        wq_c = _prelayout(
            (Wq_h[:, heads_q, :].reshape(HID, 512) * SCALE).astype(NPBF16), 512)
        bq_c = (bq_h[heads_q].reshape(512) * SCALE).reshape(4, 128).T
        wk_c = Wkv[:, 128 * r:128 * (r + 1)].astype(NPBF16)
        bk_c = bkv[128 * r:128 * (r + 1)]
        wv_c = Wkv[:, 512 + 128 * r:512 + 128 * (r + 1)].astype(NPBF16)
        # wkv interleave: chunk k -> [wk_k | wv_k]
        wkv_c = np.ascontiguousarray(np.concatenate(
            [np.concatenate([wk_c.reshape(NKC, 128, 128)[k],
                             wv_c.reshape(NKC, 128, 128)[k]], axis=1)
             for k in range(NKC)], axis=1))  # [128, NKC*256]
        bv_c = bkv[512 + 128 * r:512 + 128 * (r + 1)]
        # wo: rows = own heads' dims pair-major; cols = all 2048 features
        wo_c = np.ascontiguousarray(np.concatenate(
            [np.concatenate([Wo_h[8 * r + pr], Wo_h[8 * r + pr + 4]], axis=0)
             for pr in range(NPAIR)], axis=1).astype(NPBF16))  # [128, 4*2048]
        cbf = np.zeros((128, 640), NPBF16)
        cbf[:, 0:128] = ones_h
        cbf[0, 128:640] = np.tile(bv_c, 4).astype(NPBF16)
        cf = np.zeros((128, 21), np.float32)
        cf[:, 0:4] = bq_c
        cf[:, 4] = bk_c
        cf[:, 5 + 4 * r:5 + 4 * (r + 1)] = \
            bo[512 * r:512 * (r + 1)].reshape(4, 128).T
        in_maps.append({
            "qT": qTh[b], "kvT": kvTh[b],
            "wq": wq_c, "wkv": wkv_c, "wo": wo_c,
            "consts_bf": np.ascontiguousarray(cbf),
            "consts_f32": np.ascontiguousarray(cf),
            "band": bandh[b],
        })

    last_results = run_bass_kernel_spmd(nc, in_maps, core_ids=list(range(N_CORES)))

    out = np.empty((B, L, HID), np.float32)
    for c in range(N_CORES):
        b, r = c // TPR, c % TPR
        for j in range(NJ):
            out[b, 512 * j:512 * (j + 1), 512 * r:512 * (r + 1)] = \
                last_results.results[c][f"out{j}"].T.astype(np.float32)
    return out

